# revision 6
# baseline (speedup 1.0000x reference)
"""Complex 2x2 nearest-neighbor upsampling on 8 Trainium2 NeuronCores.

out[b, i, j, c] = complex(x_re, x_im)[b, i//2, j//2, c]

Full shapes: x_re/x_im f32 [16, 128, 128, 64] -> out complex64 [16, 256, 256, 64].

Strategy (pure data movement, memory-bound):
  - Batch-parallel: 2 images per core (16 / 8).
  - SBUF layout: partition p = input row h, free dim = a chunk of WC input
    pixels x 64 channels. Input DMAs are [128 x 8KB-contiguous] reads.
  - DVE builds the fully interleaved, width-duplicated output chunk in SBUF:
    free dim (w, dup_w, c, re/im). 4 strided copies per chunk.
  - Each SBUF output chunk is DMA'd to HBM twice (duplicate output rows
    2h and 2h+1), each DMA [128 partitions x 32KB contiguous] = 4 MiB.
  - Raw bass pipeline: ACT issues loads, DVE interleaves, SP issues stores.
    Standalone wait instructions (the direct2d DMA encoding only allows a
    single inline wait).
  - Host just views the f32 [.., 64, 2] output as complex64 and concatenates.
"""

import numpy as np

import concourse.bass as bass
import concourse.mybir as mybir
from concourse.bass_utils import run_bass_kernel_spmd

N_CORES = 8
B_FULL = 16
B = B_FULL // N_CORES  # images per core
H = 128
W = 128
C = 64
HO = 2 * H
WO = 2 * W
WC = 32  # input pixels per chunk
NCHUNK = W // WC
NITER = B * NCHUNK
IN_BUFS = 3
OUT_BUFS = 2

_cached = None


def build_nc():
    nc = bass.Bass()
    x_re = nc.dram_tensor("x_re", [B, H, W, C], mybir.dt.float32, kind="ExternalInput")
    x_im = nc.dram_tensor("x_im", [B, H, W, C], mybir.dt.float32, kind="ExternalInput")
    # f32 view of the complex64 output: last dim interleaves (c, re/im)
    out = nc.dram_tensor(
        "out", [B, HO, WO, 2 * C], mybir.dt.float32, kind="ExternalOutput"
    )

    f32 = mybir.dt.float32

    def in_src(x, i):
        b, w0 = divmod(i, NCHUNK)
        w0 *= WC
        return x[b, :, w0 : w0 + WC, :].rearrange("h w c -> h (w c)")

    def out_dst(i, dh):
        b, w0 = divmod(i, NCHUNK)
        w0 *= WC
        ob = out[b].rearrange("(h two) wo cr -> h two (wo cr)", two=2)
        return ob[:, dh, 2 * w0 * 2 * C : 2 * (w0 + WC) * 2 * C]

    from contextlib import ExitStack

    with (
        ExitStack() as stack,
        nc.semaphore() as s_load,
        nc.semaphore() as s_copy,
        nc.semaphore() as s_out,
        nc.Block() as block,
    ):
        t_re = [
            stack.enter_context(nc.sbuf_tensor(f"t_re{j}", [H, WC * C], f32))
            for j in range(IN_BUFS)
        ]
        t_im = [
            stack.enter_context(nc.sbuf_tensor(f"t_im{j}", [H, WC * C], f32))
            for j in range(IN_BUFS)
        ]
        t_out = [
            stack.enter_context(nc.sbuf_tensor(f"t_out{j}", [H, WC * 2 * C * 2], f32))
            for j in range(OUT_BUFS)
        ]

        @block.scalar
        def _(scalar):
            for i in range(NITER):
                s = i % IN_BUFS
                if i >= IN_BUFS:
                    # copies of iter i-IN_BUFS have finished reading this slot
                    scalar.wait_ge(s_copy, 4 * (i - IN_BUFS + 1))
                scalar.dma_start(out=t_re[s][:, :], in_=in_src(x_re, i)).then_inc(s_load, 16)
                scalar.dma_start(out=t_im[s][:, :], in_=in_src(x_im, i)).then_inc(s_load, 16)

        @block.vector
        def _(vector):
            for i in range(NITER):
                s = i % IN_BUFS
                so = i % OUT_BUFS
                vector.wait_ge(s_load, 32 * (i + 1))
                if i >= OUT_BUFS:
                    # stores of iter i-OUT_BUFS have finished reading this slot
                    vector.wait_ge(s_out, 32 * (i - OUT_BUFS + 1))
                ov = t_out[so][:, :].rearrange(
                    "p (w dk c ri) -> p w dk c ri", w=WC, dk=2, c=C, ri=2
                )
                ir = t_re[s][:, :].rearrange("p (w c) -> p w c", w=WC)
                ii = t_im[s][:, :].rearrange("p (w c) -> p w c", w=WC)
                vector.tensor_copy(ov[:, :, 0, :, 0], ir).then_inc(s_copy, 1)
                vector.tensor_copy(ov[:, :, 1, :, 0], ir).then_inc(s_copy, 1)
                vector.tensor_copy(ov[:, :, 0, :, 1], ii).then_inc(s_copy, 1)
                vector.tensor_copy(ov[:, :, 1, :, 1], ii).then_inc(s_copy, 1)

        @block.sync
        def _(sync):
            for i in range(NITER):
                so = i % OUT_BUFS
                sync.wait_ge(s_copy, 4 * (i + 1))
                flat = t_out[so][:, :]
                sync.dma_start(out=out_dst(i, 0), in_=flat).then_inc(s_out, 16)
                sync.dma_start(out=out_dst(i, 1), in_=flat).then_inc(s_out, 16)

    return nc


def kernel(x_re: np.ndarray, x_im: np.ndarray) -> np.ndarray:
    global _cached
    if _cached is None:
        _cached = build_nc()
    nc = _cached

    x_re = np.asarray(x_re, dtype=np.float32)
    x_im = np.asarray(x_im, dtype=np.float32)

    in_maps = [
        {
            "x_re": np.ascontiguousarray(x_re[B * c : B * (c + 1)]),
            "x_im": np.ascontiguousarray(x_im[B * c : B * (c + 1)]),
        }
        for c in range(N_CORES)
    ]
    res = run_bass_kernel_spmd(nc, in_maps, core_ids=list(range(N_CORES)))
    parts = [
        np.ascontiguousarray(r["out"]).view(np.complex64).reshape(B, HO, WO, C)
        for r in res.results
    ]
    return np.concatenate(parts, axis=0)


# revision 7
# speedup vs baseline: 139.8877x; 139.8877x over previous
"""Complex 2x2 nearest-neighbor upsampling on 8 Trainium2 NeuronCores.

out[b, i, j, c] = complex(x_re, x_im)[b, i//2, j//2, c]

Full shapes: x_re/x_im f32 [16, 128, 128, 64] -> out complex64 [16, 256, 256, 64].

Strategy (pure data movement, memory-bound):
  - Batch-parallel: 2 images per core (16 / 8).
  - SBUF layout: partition p = input row h, free dim = a chunk of WC input
    pixels x 64 channels. Input DMAs are [128 x 8KB-contiguous] reads.
  - DVE builds the fully interleaved, width-duplicated output chunk in SBUF:
    free dim (w, dup_w, c, re/im). 4 strided copies per chunk.
  - Each SBUF output chunk is DMA'd to HBM twice (duplicate output rows
    2h and 2h+1), each DMA [128 partitions x 32KB contiguous] = 4 MiB.
  - Raw bass pipeline: ACT issues loads, DVE interleaves, SP issues stores.
    Standalone wait instructions (the direct2d DMA encoding only allows a
    single inline wait).
  - Host just views the f32 [.., 64, 2] output as complex64 and concatenates.
"""

import numpy as np

import concourse.bass as bass
import concourse.mybir as mybir
from concourse.bass_utils import run_bass_kernel_spmd

N_CORES = 8
B_FULL = 16
B = B_FULL // N_CORES  # images per core
H = 128
W = 128
C = 64
HO = 2 * H
WO = 2 * W
WC = 32  # input pixels per chunk
NCHUNK = W // WC
NITER = B * NCHUNK
IN_BUFS = 3
OUT_BUFS = 2

_cached = None


def build_nc(reps: int = 1):
    nc = bass.Bass()
    x_re = nc.dram_tensor("x_re", [B, H, W, C], mybir.dt.float32, kind="ExternalInput")
    x_im = nc.dram_tensor("x_im", [B, H, W, C], mybir.dt.float32, kind="ExternalInput")
    # f32 view of the complex64 output: last dim interleaves (c, re/im)
    out = nc.dram_tensor(
        "out", [B, HO, WO, 2 * C], mybir.dt.float32, kind="ExternalOutput"
    )

    f32 = mybir.dt.float32

    def in_src(x, i):
        b, w0 = divmod(i % NITER, NCHUNK)
        w0 *= WC
        return x[b, :, w0 : w0 + WC, :].rearrange("h w c -> h (w c)")

    def out_dst(i, dh):
        b, w0 = divmod(i % NITER, NCHUNK)
        w0 *= WC
        ob = out[b].rearrange("(h two) wo cr -> h two (wo cr)", two=2)
        return ob[:, dh, 2 * w0 * 2 * C : 2 * (w0 + WC) * 2 * C]

    from contextlib import ExitStack

    with (
        ExitStack() as stack,
        nc.semaphore() as s_load,
        nc.semaphore() as s_copy,
        nc.semaphore() as s_out,
        nc.Block() as block,
    ):
        t_re = [
            stack.enter_context(nc.sbuf_tensor(f"t_re{j}", [H, WC * C], f32))
            for j in range(IN_BUFS)
        ]
        t_im = [
            stack.enter_context(nc.sbuf_tensor(f"t_im{j}", [H, WC * C], f32))
            for j in range(IN_BUFS)
        ]
        t_out = [
            stack.enter_context(nc.sbuf_tensor(f"t_out{j}", [H, WC * 2 * C * 2], f32))
            for j in range(OUT_BUFS)
        ]

        @block.scalar
        def _(scalar):
            for i in range(reps * NITER):
                s = i % IN_BUFS
                if i >= IN_BUFS:
                    # copies of iter i-IN_BUFS have finished reading this slot
                    scalar.wait_ge(s_copy, 4 * (i - IN_BUFS + 1))
                scalar.dma_start(out=t_re[s][:, :], in_=in_src(x_re, i)).then_inc(s_load, 16)
                scalar.dma_start(out=t_im[s][:, :], in_=in_src(x_im, i)).then_inc(s_load, 16)

        @block.vector
        def _(vector):
            for i in range(reps * NITER):
                s = i % IN_BUFS
                so = i % OUT_BUFS
                vector.wait_ge(s_load, 32 * (i + 1))
                if i >= OUT_BUFS:
                    # stores of iter i-OUT_BUFS have finished reading this slot
                    vector.wait_ge(s_out, 32 * (i - OUT_BUFS + 1))
                ov = t_out[so][:, :].rearrange(
                    "p (w dk c ri) -> p w dk c ri", w=WC, dk=2, c=C, ri=2
                )
                ir = t_re[s][:, :].rearrange("p (w c) -> p w c", w=WC)
                ii = t_im[s][:, :].rearrange("p (w c) -> p w c", w=WC)
                vector.tensor_copy(ov[:, :, 0, :, 0], ir).then_inc(s_copy, 1)
                vector.tensor_copy(ov[:, :, 1, :, 0], ir).then_inc(s_copy, 1)
                vector.tensor_copy(ov[:, :, 0, :, 1], ii).then_inc(s_copy, 1)
                vector.tensor_copy(ov[:, :, 1, :, 1], ii).then_inc(s_copy, 1)

        @block.sync
        def _(sync):
            for i in range(reps * NITER):
                so = i % OUT_BUFS
                sync.wait_ge(s_copy, 4 * (i + 1))
                flat = t_out[so][:, :]
                sync.dma_start(out=out_dst(i, 0), in_=flat).then_inc(s_out, 16)
                sync.dma_start(out=out_dst(i, 1), in_=flat).then_inc(s_out, 16)

    return nc


def kernel(x_re: np.ndarray, x_im: np.ndarray) -> np.ndarray:
    global _cached
    if _cached is None:
        _cached = build_nc()
    nc = _cached

    x_re = np.asarray(x_re, dtype=np.float32)
    x_im = np.asarray(x_im, dtype=np.float32)

    in_maps = [
        {
            "x_re": np.ascontiguousarray(x_re[B * c : B * (c + 1)]),
            "x_im": np.ascontiguousarray(x_im[B * c : B * (c + 1)]),
        }
        for c in range(N_CORES)
    ]
    res = run_bass_kernel_spmd(nc, in_maps, core_ids=list(range(N_CORES)))
    parts = [
        np.ascontiguousarray(r["out"]).view(np.complex64).reshape(B, HO, WO, C)
        for r in res.results
    ]
    return np.concatenate(parts, axis=0)


# revision 8
# speedup vs baseline: 243.6035x; 1.7414x over previous
"""Complex 2x2 nearest-neighbor upsampling on 8 Trainium2 NeuronCores.

out[b, i, j, c] = complex(x_re, x_im)[b, i//2, j//2, c]

Full shapes: x_re/x_im f32 [16, 128, 128, 64] -> out complex64 [16, 256, 256, 64].

Strategy (pure data movement, memory-bound):
  - Batch-parallel: 2 images per core (16 / 8).
  - SBUF layout: partition p = input row h, free dim = a chunk of WC input
    pixels x 64 channels. Input DMAs are [128 x 8KB-contiguous] reads.
  - DVE builds the fully interleaved, width-duplicated output chunk in SBUF:
    free dim (w, dup_w, c, re/im). 4 strided copies per chunk.
  - Each SBUF output chunk is DMA'd to HBM twice (duplicate output rows
    2h and 2h+1), each DMA [128 partitions x 32KB contiguous] = 4 MiB.
  - Raw bass pipeline: ACT issues loads, DVE interleaves, SP issues stores.
    Standalone wait instructions (the direct2d DMA encoding only allows a
    single inline wait).
  - Host just views the f32 [.., 64, 2] output as complex64 and concatenates.
"""

import numpy as np

import concourse.bass as bass
import concourse.mybir as mybir
from concourse.bass_utils import run_bass_kernel_spmd

N_CORES = 8
B_FULL = 16
B = B_FULL // N_CORES  # images per core
H = 128
W = 128
C = 64
HO = 2 * H
WO = 2 * W
WC = 32  # input pixels per chunk
NCHUNK = W // WC
NITER = B * NCHUNK
IN_BUFS = 3
OUT_BUFS = 2

_cached = None


def build_nc(reps: int = 1):
    nc = bass.Bass()
    x_re = nc.dram_tensor("x_re", [B, H, W, C], mybir.dt.float32, kind="ExternalInput")
    x_im = nc.dram_tensor("x_im", [B, H, W, C], mybir.dt.float32, kind="ExternalInput")
    # f32 view of the complex64 output: last dim interleaves (c, re/im)
    out = nc.dram_tensor(
        "out", [B, HO, WO, 2 * C], mybir.dt.float32, kind="ExternalOutput"
    )

    f32 = mybir.dt.float32

    def in_src(x, i):
        b, w0 = divmod(i % NITER, NCHUNK)
        w0 *= WC
        return x[b, :, w0 : w0 + WC, :].rearrange("h w c -> h (w c)")

    def out_dst(i, dh):
        b, w0 = divmod(i % NITER, NCHUNK)
        w0 *= WC
        ob = out[b].rearrange("(h two) wo cr -> h two (wo cr)", two=2)
        return ob[:, dh, 2 * w0 * 2 * C : 2 * (w0 + WC) * 2 * C]

    from contextlib import ExitStack

    with (
        ExitStack() as stack,
        nc.semaphore() as s_copy,
        nc.Block() as block,
    ):
        s_load = [
            stack.enter_context(nc.semaphore(f"s_load{j}")) for j in range(IN_BUFS)
        ]
        s_out = [
            stack.enter_context(nc.semaphore(f"s_out{j}")) for j in range(OUT_BUFS)
        ]
        t_re = [
            stack.enter_context(nc.sbuf_tensor(f"t_re{j}", [H, WC * C], f32))
            for j in range(IN_BUFS)
        ]
        t_im = [
            stack.enter_context(nc.sbuf_tensor(f"t_im{j}", [H, WC * C], f32))
            for j in range(IN_BUFS)
        ]
        t_out = [
            stack.enter_context(nc.sbuf_tensor(f"t_out{j}", [H, WC * 2 * C * 2], f32))
            for j in range(OUT_BUFS)
        ]

        @block.scalar
        def _(scalar):
            for i in range(reps * NITER):
                s = i % IN_BUFS
                if i >= IN_BUFS:
                    # copies of iter i-IN_BUFS have finished reading this slot
                    scalar.wait_ge(s_copy, 4 * (i - IN_BUFS + 1))
                scalar.dma_start(out=t_re[s][:, :], in_=in_src(x_re, i)).then_inc(
                    s_load[s], 16
                )
                scalar.dma_start(out=t_im[s][:, :], in_=in_src(x_im, i)).then_inc(
                    s_load[s], 16
                )

        @block.vector
        def _(vector):
            for i in range(reps * NITER):
                s = i % IN_BUFS
                so = i % OUT_BUFS
                vector.wait_ge(s_load[s], 32 * (i // IN_BUFS + 1))
                if i >= OUT_BUFS:
                    # stores of iter i-OUT_BUFS have finished reading this slot
                    vector.wait_ge(s_out[so], 32 * ((i - OUT_BUFS) // OUT_BUFS + 1))
                ov = t_out[so][:, :].rearrange(
                    "p (w dk c ri) -> p w dk c ri", w=WC, dk=2, c=C, ri=2
                )
                ir = t_re[s][:, :].rearrange("p (w c) -> p w c", w=WC)
                ii = t_im[s][:, :].rearrange("p (w c) -> p w c", w=WC)
                vector.tensor_copy(ov[:, :, 0, :, 0], ir).then_inc(s_copy, 1)
                vector.tensor_copy(ov[:, :, 1, :, 0], ir).then_inc(s_copy, 1)
                vector.tensor_copy(ov[:, :, 0, :, 1], ii).then_inc(s_copy, 1)
                vector.tensor_copy(ov[:, :, 1, :, 1], ii).then_inc(s_copy, 1)

        @block.sync
        def _(sync):
            for i in range(reps * NITER):
                so = i % OUT_BUFS
                sync.wait_ge(s_copy, 4 * (i + 1))
                flat = t_out[so][:, :]
                sync.dma_start(out=out_dst(i, 0), in_=flat).then_inc(s_out[so], 16)
                sync.dma_start(out=out_dst(i, 1), in_=flat).then_inc(s_out[so], 16)

    return nc


def kernel(x_re: np.ndarray, x_im: np.ndarray) -> np.ndarray:
    global _cached
    if _cached is None:
        _cached = build_nc()
    nc = _cached

    x_re = np.asarray(x_re, dtype=np.float32)
    x_im = np.asarray(x_im, dtype=np.float32)

    in_maps = [
        {
            "x_re": np.ascontiguousarray(x_re[B * c : B * (c + 1)]),
            "x_im": np.ascontiguousarray(x_im[B * c : B * (c + 1)]),
        }
        for c in range(N_CORES)
    ]
    res = run_bass_kernel_spmd(nc, in_maps, core_ids=list(range(N_CORES)))
    parts = [
        np.ascontiguousarray(r["out"]).view(np.complex64).reshape(B, HO, WO, C)
        for r in res.results
    ]
    return np.concatenate(parts, axis=0)


# revision 9
# speedup vs baseline: 386.0818x; 1.5849x over previous
"""Complex 2x2 nearest-neighbor upsampling on 8 Trainium2 NeuronCores.

out[b, i, j, c] = complex(x_re, x_im)[b, i//2, j//2, c]

Full shapes: x_re/x_im f32 [16, 128, 128, 64] -> out complex64 [16, 256, 256, 64].

Strategy (pure data movement, memory-bound):
  - Batch-parallel: 2 images per core (16 / 8).
  - SBUF layout: partition p = input row h, free dim = a chunk of WC input
    pixels x 64 channels. Input DMAs are [128 x 8KB-contiguous] reads.
  - DVE builds the fully interleaved, width-duplicated output chunk in SBUF:
    free dim (w, dup_w, c, re/im). 4 strided copies per chunk.
  - Each SBUF output chunk is DMA'd to HBM twice (duplicate output rows
    2h and 2h+1), each DMA [128 partitions x 32KB contiguous] = 4 MiB.
  - Raw bass pipeline: ACT issues loads, DVE interleaves, SP issues stores.
    Standalone wait instructions (the direct2d DMA encoding only allows a
    single inline wait).
  - Host just views the f32 [.., 64, 2] output as complex64 and concatenates.
"""

import numpy as np

import concourse.bass as bass
import concourse.mybir as mybir
from concourse.bass_utils import run_bass_kernel_spmd

N_CORES = 8
B_FULL = 16
B = B_FULL // N_CORES  # images per core
H = 128
W = 128
C = 64
HO = 2 * H
WO = 2 * W
WC = 32  # input pixels per chunk
NCHUNK = W // WC
NITER = B * NCHUNK
IN_BUFS = 3
OUT_BUFS = 2

_cached = None


def build_nc(reps: int = 1):
    nc = bass.Bass()
    x_re = nc.dram_tensor("x_re", [B, H, W, C], mybir.dt.float32, kind="ExternalInput")
    x_im = nc.dram_tensor("x_im", [B, H, W, C], mybir.dt.float32, kind="ExternalInput")
    # f32 view of the complex64 output: last dim interleaves (c, re/im)
    out = nc.dram_tensor(
        "out", [B, HO, WO, 2 * C], mybir.dt.float32, kind="ExternalOutput"
    )

    f32 = mybir.dt.float32

    def in_src(x, i):
        b, w0 = divmod(i % NITER, NCHUNK)
        w0 *= WC
        return x[b, :, w0 : w0 + WC, :].rearrange("h w c -> h (w c)")

    def out_dst(i, dh):
        b, w0 = divmod(i % NITER, NCHUNK)
        w0 *= WC
        ob = out[b].rearrange("(h two) wo cr -> h two (wo cr)", two=2)
        return ob[:, dh, 2 * w0 * 2 * C : 2 * (w0 + WC) * 2 * C]

    from contextlib import ExitStack

    with (
        ExitStack() as stack,
        nc.semaphore() as s_copy,
        nc.Block() as block,
    ):
        s_load = [
            stack.enter_context(nc.semaphore(f"s_load{j}")) for j in range(IN_BUFS)
        ]
        s_out = [
            stack.enter_context(nc.semaphore(f"s_out{j}")) for j in range(OUT_BUFS)
        ]
        t_re = [
            stack.enter_context(nc.sbuf_tensor(f"t_re{j}", [H, WC * C], f32))
            for j in range(IN_BUFS)
        ]
        t_im = [
            stack.enter_context(nc.sbuf_tensor(f"t_im{j}", [H, WC * C], f32))
            for j in range(IN_BUFS)
        ]
        t_out = [
            stack.enter_context(nc.sbuf_tensor(f"t_out{j}", [H, WC * 2 * C * 2], f32))
            for j in range(OUT_BUFS)
        ]

        @block.gpsimd
        def _(gpsimd):
            for i in range(reps * NITER):
                s = i % IN_BUFS
                if i >= IN_BUFS:
                    # copies of iter i-IN_BUFS have finished reading this slot
                    gpsimd.wait_ge(s_copy, 4 * (i - IN_BUFS + 1))
                gpsimd.dma_start(out=t_re[s][:, :], in_=in_src(x_re, i)).then_inc(
                    s_load[s], 16
                )
                gpsimd.dma_start(out=t_im[s][:, :], in_=in_src(x_im, i)).then_inc(
                    s_load[s], 16
                )

        @block.vector
        def _(vector):
            for i in range(reps * NITER):
                s = i % IN_BUFS
                so = i % OUT_BUFS
                vector.wait_ge(s_load[s], 32 * (i // IN_BUFS + 1))
                if i >= OUT_BUFS:
                    # stores of iter i-OUT_BUFS have finished reading this slot
                    vector.wait_ge(s_out[so], 32 * ((i - OUT_BUFS) // OUT_BUFS + 1))
                ov = t_out[so][:, :].rearrange(
                    "p (w dk c ri) -> p w dk c ri", w=WC, dk=2, c=C, ri=2
                )
                ir = t_re[s][:, :].rearrange("p (w c) -> p w c", w=WC)
                ii = t_im[s][:, :].rearrange("p (w c) -> p w c", w=WC)
                vector.tensor_copy(ov[:, :, 0, :, 0], ir).then_inc(s_copy, 1)
                vector.tensor_copy(ov[:, :, 1, :, 0], ir).then_inc(s_copy, 1)
                vector.tensor_copy(ov[:, :, 0, :, 1], ii).then_inc(s_copy, 1)
                vector.tensor_copy(ov[:, :, 1, :, 1], ii).then_inc(s_copy, 1)

        @block.sync
        def _(sync):
            for i in range(reps * NITER):
                so = i % OUT_BUFS
                sync.wait_ge(s_copy, 4 * (i + 1))
                sync.dma_start(out=out_dst(i, 0), in_=t_out[so][:, :]).then_inc(
                    s_out[so], 16
                )

        @block.scalar
        def _(scalar):
            for i in range(reps * NITER):
                so = i % OUT_BUFS
                scalar.wait_ge(s_copy, 4 * (i + 1))
                scalar.dma_start(out=out_dst(i, 1), in_=t_out[so][:, :]).then_inc(
                    s_out[so], 16
                )

    return nc


def kernel(x_re: np.ndarray, x_im: np.ndarray) -> np.ndarray:
    global _cached
    if _cached is None:
        _cached = build_nc()
    nc = _cached

    x_re = np.asarray(x_re, dtype=np.float32)
    x_im = np.asarray(x_im, dtype=np.float32)

    in_maps = [
        {
            "x_re": np.ascontiguousarray(x_re[B * c : B * (c + 1)]),
            "x_im": np.ascontiguousarray(x_im[B * c : B * (c + 1)]),
        }
        for c in range(N_CORES)
    ]
    res = run_bass_kernel_spmd(nc, in_maps, core_ids=list(range(N_CORES)))
    parts = [
        np.ascontiguousarray(r["out"]).view(np.complex64).reshape(B, HO, WO, C)
        for r in res.results
    ]
    return np.concatenate(parts, axis=0)
